# revision 17
# baseline (speedup 1.0000x reference)
"""Per-frame RMS energy (STFT framing: n_fft=1024, hop=256, center/reflect pad)
over a [16, 1048576] f32 signal -> [16, 4096, 1] f32.

Trainium2 Bass/Tile kernel, data-parallel over batch across 8 NeuronCores
(2 signals per core). Each 1024-sample frame is exactly 4 consecutive
256-sample hop blocks, so we compute per-block sums of squares (one read of
every input byte -> memory-bound optimal), then a sliding sum of 4 plus
sqrt(mean).

Layout: partition p of a signal owns frames p*32..p*32+31; its input row is
the naturally aligned x[p*8192 : (p+1)*8192]. Both signals share ONE
"extended block sums" tile ext01[128, 72]: cols 0..34 = s_pad[32p+u] for
signal 0, cols 36..70 for signal 1, so the window adds / sqrt / outputs run
once over the pair instead of twice.

v6 design (vs the 41.9us baseline):
- Full SBUF residency: all 16 bulk triggers issue back-to-back, the HW
  queue streams at its ~400 GB/s ceiling (the baseline ran compute-paced
  at ~256 GB/s through an 8-buffer ring).
- Total compute (square every sample + block-sum; measured ACT 1.12ns/elem,
  DVE reduce 1.19ns/elem — no fast mode exists for TensorReduce — and Pool
  TensorTensor at 0.42 efficiency) is ~48us of engine time against a ~21us
  stream, so it is spread across three engines, each kept at <=~19us:
  - ACT squares five 1024-sample chunks per signal; Pool squares one and
    pairwise-halves four, so DVE's grouped reduce sees half the elements
    there; DVE does 4 half-reduces + 2 full reduces per signal.
  - The last EIGHT blocks of each signal are single-block fused
    square+accum ops (ACT activation accum_out 767ns / DVE stt accum
    504ns) with no cross-engine chain, so nothing that lands in the last
    ~4us of the stream needs a square->reduce pipeline, and the seam
    sources (blocks 30/31) are fused columns available mid-tail.
  - Window adds run on Pool (idle after ~25us), sqrt on ACT, outputs on
    Sync's ring (idle after its triggers).
- Reflect-pad edges come from resident data: s_pad[0] via fused ACT
  square+accum on partition 0; s_pad[1] / s_pad[4098] via full-partition
  fused stt accumulating straight into ext columns 1 / 34 (junk rows are
  overwritten by the later-emitted seam DMAs; the real row survives).
- Chunk arrival order interleaves signals and paths so ACT, Pool and DVE
  all have work from the first arrivals.
"""

import sys
import types

import numpy as np

import concourse.bacc as bacc
import concourse.bass as bass
import concourse.mybir as mybir
import concourse.tile as tile
from concourse.bass_utils import run_bass_kernel_spmd
from concourse.vector_clock import ScopedClock


def _install_ntff_hook_shim():
    """The image's antenv lacks axon_hooks; if a caller turns on tracing
    (e.g. via BASS_TRACE=1), run_bass_kernel_spmd imports it. Provide the
    ctypes-based hook so that path works instead of raising."""
    try:
        import antenv.axon_hooks  # noqa: F401

        return
    except ImportError:
        pass
    try:
        from trn_agent_boot.trn_boot import _ntff_profile_via_ctypes

        hook = _ntff_profile_via_ctypes("/opt/axon/libaxon_pjrt.so")
    except Exception:
        hook = None
    mod = types.ModuleType("antenv.axon_hooks")
    mod.get_axon_ntff_profile_hook = lambda: hook
    mod.set_axon_ntff_profile_hook = lambda h: None
    sys.modules["antenv.axon_hooks"] = mod


_install_ntff_hook_shim()


class SlimExitTileContext(tile.TileContext):
    """TileContext whose exit sequence drops the second all-engine barrier.

    The stock epilogue is drain -> barrier -> sem clear -> barrier. The
    first barrier guarantees every engine is idle before the gpsimd range
    sem-clear runs; the trailing barrier only re-synchronizes engines that
    are each about to run off the end of their own queues, so skipping it
    is safe (NRT completion still waits for every queue, and the sem state
    a re-execution needs is restored by the clear).
    """

    def _drain_and_barrier(self, tick_clock, wait_clock):
        drain_inst = self.nc.gpsimd.drain()
        wait_clock.add_sem_waits(
            drain_inst.ins, ScopedClock({None: tick_clock.global_clock})
        )
        assert self.sems is not None
        popped = self.nc._tile_sem_poison_stack.pop()
        assert popped is self._sem_poison
        self.nc.clear_and_free_semaphores(list(self.sems.allocated().values()))

# Problem constants (self-contained; must match the grader's input spec)
B = 16                 # signals in the batch
T = 1048576            # samples per signal
N_FFT = 1024
HOP = 256
N_CORES = 8
SIG_PER_CORE = B // N_CORES   # 2
P = 128                       # SBUF partitions
NBLK = T // HOP               # 4096 hop blocks per signal
CPB = NBLK // P               # 32 output frames per partition
NFRAMES = NBLK                # 4096 output frames per signal

F32 = mybir.dt.float32
AF = mybir.ActivationFunctionType
AX = mybir.AxisListType
ADD = mybir.AluOpType.add
MULT = mybir.AluOpType.mult

# ext01 column base per signal
EXTBASE = [0, 36]

# Per-signal chunk table: name -> (block_offset, n_blocks).
# h0: Pool-squared + Pool-halved (ledge source); h1..h3: ACT-squared +
# Pool-halved; h4, h5: ACT-squared + DVE full reduce; f1 (28,4) per-block
# fused (holds the seam + right-edge sources — it arrives MID-stream so the
# slow partition-shift seam DMAs, ~2.2us for 127 tiny packets, complete
# well before the window adds); f0a/f0b: tiny fused tail chunks.
CHUNKS = {
    "h0": (0, 4),
    "h1": (4, 4),
    "h2": (8, 4),
    "h3": (12, 4),
    "h4": (16, 4),
    "h5": (20, 4),
    "f1": (28, 4),
    "f0a": (24, 2),
    "f0b": (26, 2),
}

# Global arrival order: (sig, chunk). First rides the ACT HWDGE ring (its
# queue opens ~1us before Sync clears its tile-entry drain); the rest ride
# Sync's ring back-to-back.
ARRIVALS = [
    (0, "h1"),   # ACT's first square asap
    (0, "h0"),   # Pool's square + ledge sources
    (0, "h4"),   # DVE's first full reduce
    (1, "h1"),
    (1, "h0"),
    (0, "h2"),
    (1, "h4"),
    (0, "f1"),   # seam sources mid-stream
    (1, "f1"),
    (0, "h5"),
    (1, "h2"),
    (0, "h3"),
    (1, "h5"),
    (1, "h3"),
    (0, "f0a"), (1, "f0a"),
    (0, "f0b"), (1, "f0b"),
]


def build_bass():
    # Bacc (not plain Bass): its compile pipeline splits multi-sem waits
    # into event-semaphore instructions, which this walrus build requires.
    # Bass.__init__ ends with an all-engine barrier whose only job is to
    # order its const-AP memsets against const-AP readers; this kernel
    # reads no const APs (explicit zeros-tile bias), so skip it.
    orig_barrier = bass.Bass.all_engine_barrier
    bass.Bass.all_engine_barrier = lambda self, *, sem_only=False: None
    try:
        nc = bacc.Bacc()
    finally:
        bass.Bass.all_engine_barrier = orig_barrier
    x = nc.dram_tensor("signal", [SIG_PER_CORE, T], F32, kind="ExternalInput")
    y = nc.dram_tensor("out", [SIG_PER_CORE, NFRAMES], F32, kind="ExternalOutput")

    xr = x[:, :].rearrange("b (p f) -> b p f", p=P)   # [2, 128, 8192]
    yr = y[:, :].rearrange("b (p c) -> b p c", p=P)   # [2, 128, 32]

    with SlimExitTileContext(nc) as tc:
        with (
            tc.tile_pool(name="inp", bufs=1) as inp_pool,
            tc.tile_pool(name="sq", bufs=1) as sq_pool,
            tc.tile_pool(name="ext", bufs=1) as ext_pool,
            tc.tile_pool(name="spec", bufs=1) as spec_pool,
            tc.tile_pool(name="small", bufs=1) as small_pool,
        ):
            # --- loads -------------------------------------------------
            tins = {}
            for i, (sig, cn) in enumerate(ARRIVALS):
                b0, nb = CHUNKS[cn]
                t = inp_pool.tile([P, nb * HOP], F32, tag=f"tin_{sig}_{cn}")
                eng = nc.scalar if i == 0 else nc.sync
                eng.dma_start(
                    out=t[:, :], in_=xr[sig, :, b0 * HOP : b0 * HOP + nb * HOP]
                )
                tins[(sig, cn)] = t

            # Zeros tile as the explicit activation bias; dummy Sqrt
            # preloads the ACT table set covering Square+Sqrt up front.
            # Memsets on GpSimd (Memset efficiency 1.0, frees DVE time).
            zb = spec_pool.tile([P, 1], F32, tag="zb")
            nc.gpsimd.memset(zb[:, :], 0.0)
            dummy = spec_pool.tile([1, 1], F32, tag="dummy")
            nc.gpsimd.memset(dummy[0:1, 0:1], 1.0)
            nc.scalar.activation(
                out=dummy[0:1, 0:1], in_=dummy[0:1, 0:1], func=AF.Sqrt,
                bias=zb[0:1, 0:1],
            )

            # Shared block-sum tile for both signals + junk-gap memset
            # (col 35 is read by the windowing as padding; col 71 unused).
            ext = ext_pool.tile([P, 72], F32, tag="ext01")
            nc.gpsimd.memset(ext[:, 35:36], 0.0)

            tsq = {}

            def col(sig, blk):
                return EXTBASE[sig] + 2 + blk

            def act_square(sig, cn):
                b0, nb = CHUNKS[cn]
                t = sq_pool.tile([P, nb * HOP], F32, tag=f"tsq_{sig}_{cn}")
                nc.scalar.activation(
                    out=t[:, :], in_=tins[(sig, cn)][:, :], func=AF.Square,
                    bias=zb[:, 0:1],
                )
                tsq[(sig, cn)] = t

            def pool_square(sig, cn):
                b0, nb = CHUNKS[cn]
                t = sq_pool.tile([P, nb * HOP], F32, tag=f"tsq_{sig}_{cn}")
                tin = tins[(sig, cn)]
                nc.gpsimd.tensor_tensor(
                    out=t[:, :], in0=tin[:, :], in1=tin[:, :], op=MULT
                )
                tsq[(sig, cn)] = t

            def pool_halve(sig, cn):
                b0, nb = CHUNKS[cn]
                hv = sq_pool.tile([P, nb * 128], F32, tag=f"hv_{sig}_{cn}")
                v = tsq[(sig, cn)][:, :].rearrange(
                    "p (g j k) -> p g j k", g=nb, j=2
                )
                nc.gpsimd.tensor_tensor(
                    out=hv[:, :].rearrange("p (g k) -> p g k", g=nb),
                    in0=v[:, :, 0, :], in1=v[:, :, 1, :], op=ADD,
                )
                tsq[(sig, cn, "hv")] = hv

            def dve_halfred(sig, cn):
                b0, nb = CHUNKS[cn]
                nc.vector.tensor_reduce(
                    out=ext[:, col(sig, b0) : col(sig, b0) + nb],
                    in_=tsq[(sig, cn, "hv")][:, :].rearrange(
                        "p (g k) -> p g k", g=nb
                    ),
                    axis=AX.X, op=ADD,
                )

            def dve_fullred(sig, cn):
                b0, nb = CHUNKS[cn]
                nc.vector.tensor_reduce(
                    out=ext[:, col(sig, b0) : col(sig, b0) + nb],
                    in_=tsq[(sig, cn)][:, :].rearrange("p (g k) -> p g k", k=HOP),
                    axis=AX.X, op=ADD,
                )

            def fused_block(sig, cn, k, eng):
                b0, nb = CHUNKS[cn]
                tin = tins[(sig, cn)]
                c = col(sig, b0 + k)
                sl = tin[:, k * HOP : (k + 1) * HOP]
                acc = ext[:, c : c + 1]
                if eng == "act":
                    scr = sq_pool.tile([P, HOP], F32, tag="scr_act", bufs=2)
                    nc.scalar.activation(
                        out=scr[:, :], in_=sl, func=AF.Square,
                        bias=zb[:, 0:1], accum_out=acc,
                    )
                else:
                    scr = sq_pool.tile([P, HOP], F32, tag="scr_dve", bufs=2)
                    nc.vector.scalar_tensor_tensor(
                        out=scr[:, :], in0=sl, scalar=1.0, in1=sl,
                        op0=MULT, op1=MULT, accum_out=acc,
                    )

            def left_edges(sig):
                # s_pad[0] -> ext col 0: fused ACT square+accum, partition 0
                # only. s_pad[1] -> ext col 1: full-partition fused stt —
                # rows 1..127 junk, overwritten by the later-emitted seam-1
                # DMA; row 0 keeps the edge sum.
                tin = tins[(sig, "h0")]
                cb = EXTBASE[sig]
                scr = sq_pool.tile([P, HOP], F32, tag="scr_act", bufs=2)
                nc.scalar.activation(
                    out=scr[0:1, :], in_=tin[0:1, 257:513], func=AF.Square,
                    bias=zb[0:1, 0:1], accum_out=ext[0:1, cb : cb + 1],
                )
                lsq = sq_pool.tile([P, HOP], F32, tag="scr_dve", bufs=2)
                nc.vector.scalar_tensor_tensor(
                    out=lsq[:, :], in0=tin[:, 1:257], scalar=1.0,
                    in1=tin[:, 1:257], op0=MULT, op1=MULT,
                    accum_out=ext[:, cb + 1 : cb + 2],
                )

            def right_edge_and_seams(sig):
                # s_pad[4098]: partition 127 of f1, local cols 767..1022
                # (x[T-257:T-1]). Full-partition fused stt into ext col 34 —
                # rows 0..126 junk, overwritten by seam-2 below.
                tin = tins[(sig, "f1")]
                cb = EXTBASE[sig]
                rsq = sq_pool.tile([P, HOP], F32, tag="scr_dve", bufs=2)
                nc.vector.scalar_tensor_tensor(
                    out=rsq[:, :], in0=tin[:, 767:1023], scalar=1.0,
                    in1=tin[:, 767:1023], op0=MULT, op1=MULT,
                    accum_out=ext[:, cb + 34 : cb + 35],
                )
                # seam1: ext[p, 0:2] = ext[p-1, 32:34]; seam2: ext[p, 34] =
                # ext[p+1, 2] (rows 0..126). On Sync's ring (idle after the
                # bulk triggers).
                nc.sync.dma_start(
                    out=ext[1:128, cb : cb + 2], in_=ext[0:127, cb + 32 : cb + 34]
                )
                nc.sync.dma_start(
                    out=ext[0:127, cb + 34 : cb + 35], in_=ext[1:128, cb + 2 : cb + 3]
                )

            # --- compute, in estimated input-readiness order -----------
            act_square(0, "h1")          # data ~9.3 (ACT ring)
            pool_square(0, "h0")         # data ~10.6
            left_edges(0)
            act_square(0, "h4")          # data ~11.9
            pool_halve(0, "h1")
            dve_halfred(0, "h1")
            act_square(1, "h1")          # data ~13.2
            pool_halve(0, "h0")
            dve_halfred(0, "h0")
            dve_fullred(0, "h4")
            pool_square(1, "h0")         # data ~14.5
            left_edges(1)
            act_square(0, "h2")          # data ~15.8
            pool_halve(1, "h1")
            dve_halfred(1, "h1")
            act_square(1, "h4")          # data ~17.1
            pool_halve(1, "h0")
            dve_halfred(1, "h0")
            # f1 fused (data s0 ~18.4, s1 ~19.7): seam sources (blocks
            # 30/31) first; seams + right edge right after, so the slow
            # partition-shift copies finish mid-stream.
            pool_halve(0, "h2")
            dve_halfred(0, "h2")
            dve_fullred(1, "h4")
            fused_block(0, "f1", 2, "dve")   # block 30
            fused_block(0, "f1", 3, "dve")   # block 31
            fused_block(0, "f1", 0, "act")   # block 28
            fused_block(0, "f1", 1, "dve")   # block 29
            right_edge_and_seams(0)
            fused_block(1, "f1", 2, "dve")
            fused_block(1, "f1", 3, "dve")
            fused_block(1, "f1", 0, "act")
            fused_block(1, "f1", 1, "dve")
            right_edge_and_seams(1)
            act_square(0, "h5")          # data ~21
            dve_fullred(0, "h5")
            act_square(1, "h2")          # data ~22.3
            pool_halve(1, "h2")
            dve_halfred(1, "h2")
            act_square(0, "h3")          # data ~23.6
            pool_halve(0, "h3")
            dve_halfred(0, "h3")
            act_square(1, "h5")          # data ~24.9
            dve_fullred(1, "h5")
            act_square(1, "h3")          # data ~26.2
            pool_halve(1, "h3")
            dve_halfred(1, "h3")
            # tiny fused tails (data ~27.5 .. 29.2), one block per engine
            fused_block(0, "f0a", 0, "act")   # block 24
            fused_block(0, "f0a", 1, "dve")   # block 25
            fused_block(1, "f0a", 0, "act")
            fused_block(1, "f0a", 1, "dve")
            fused_block(0, "f0b", 0, "act")   # block 26
            fused_block(0, "f0b", 1, "dve")   # block 27
            fused_block(1, "f0b", 0, "act")
            fused_block(1, "f0b", 1, "dve")

            # Combined windows on Pool (idle by now): p1/e over the whole
            # 72-wide pair (junk in the 2-col gap, never read downstream),
            # one sqrt on ACT, outputs on Sync.
            p1 = small_pool.tile([P, 70], F32, tag="p1")
            e1 = small_pool.tile([P, 68], F32, tag="e1")
            nc.gpsimd.tensor_tensor(
                out=p1[:, :], in0=ext[:, 0:70], in1=ext[:, 1:71], op=ADD
            )
            nc.gpsimd.tensor_tensor(
                out=e1[:, :], in0=p1[:, 0:68], in1=p1[:, 2:70], op=ADD
            )
            ot = small_pool.tile([P, 68], F32, tag="ot")
            nc.scalar.activation(
                out=ot[:, :], in_=e1[:, :], func=AF.Sqrt, scale=1.0 / N_FFT,
                bias=zb[:, 0:1],
            )
            # Outputs on two different rings so they transfer in parallel.
            nc.sync.dma_start(out=yr[0, :, :], in_=ot[:, 0:32])
            nc.scalar.dma_start(out=yr[1, :, :], in_=ot[:, 36:68])
    nc.finalize()
    return nc


_NC = None


def run(signal: np.ndarray, trace: bool = False):
    global _NC
    sig = np.ascontiguousarray(np.asarray(signal, dtype=np.float32))
    assert sig.shape == (B, T), sig.shape
    if _NC is None:
        _NC = build_bass()
    in_maps = [
        {"signal": np.ascontiguousarray(sig[k * SIG_PER_CORE : (k + 1) * SIG_PER_CORE])}
        for k in range(N_CORES)
    ]
    res = run_bass_kernel_spmd(_NC, in_maps, core_ids=list(range(N_CORES)), trace=trace)
    out = np.concatenate([r["out"] for r in res.results], axis=0)
    return out.reshape(B, NFRAMES, 1).astype(np.float32), res


def kernel(signal: np.ndarray) -> np.ndarray:
    out, _ = run(signal, trace=False)
    return out


# revision 23
# speedup vs baseline: 1.0380x; 1.0380x over previous
"""Per-frame RMS energy (STFT framing: n_fft=1024, hop=256, center/reflect pad)
over a [16, 1048576] f32 signal -> [16, 4096, 1] f32.

Trainium2 Bass/Tile kernel, data-parallel over batch across 8 NeuronCores
(2 signals per core). Each 1024-sample frame is exactly 4 consecutive
256-sample hop blocks, so we compute per-block sums of squares (one read of
every input byte -> memory-bound optimal), then a sliding sum of 4 plus
sqrt(mean).

Layout: partition p of a signal owns frames p*32..p*32+31; its input row is
the naturally aligned x[p*8192 : (p+1)*8192]. Both signals share ONE
"extended block sums" tile ext01[128, 72]: cols 0..34 = s_pad[32p+u] for
signal 0, cols 36..70 for signal 1, so the window adds / sqrt / outputs run
once over the pair instead of twice.

v6 design (vs the 41.9us baseline):
- Full SBUF residency: all 16 bulk triggers issue back-to-back, the HW
  queue streams at its ~400 GB/s ceiling (the baseline ran compute-paced
  at ~256 GB/s through an 8-buffer ring).
- Total compute (square every sample + block-sum; measured ACT 1.12ns/elem,
  DVE reduce 1.19ns/elem — no fast mode exists for TensorReduce — and Pool
  TensorTensor at 0.42 efficiency) is ~48us of engine time against a ~21us
  stream, so it is spread across three engines, each kept at <=~19us:
  - ACT squares five 1024-sample chunks per signal; Pool squares one and
    pairwise-halves four, so DVE's grouped reduce sees half the elements
    there; DVE does 4 half-reduces + 2 full reduces per signal.
  - The last EIGHT blocks of each signal are single-block fused
    square+accum ops (ACT activation accum_out 767ns / DVE stt accum
    504ns) with no cross-engine chain, so nothing that lands in the last
    ~4us of the stream needs a square->reduce pipeline, and the seam
    sources (blocks 30/31) are fused columns available mid-tail.
  - Window adds run on Pool (idle after ~25us), sqrt on ACT, outputs on
    Sync's ring (idle after its triggers).
- Reflect-pad edges come from resident data: s_pad[0] via fused ACT
  square+accum on partition 0; s_pad[1] / s_pad[4098] via full-partition
  fused stt accumulating straight into ext columns 1 / 34 (junk rows are
  overwritten by the later-emitted seam DMAs; the real row survives).
- Chunk arrival order interleaves signals and paths so ACT, Pool and DVE
  all have work from the first arrivals.
"""

import sys
import types

import numpy as np

import concourse.bacc as bacc
import concourse.bass as bass
import concourse.mybir as mybir
import concourse.tile as tile
from concourse.bass_utils import run_bass_kernel_spmd
from concourse.vector_clock import ScopedClock


def _install_ntff_hook_shim():
    """The image's antenv lacks axon_hooks; if a caller turns on tracing
    (e.g. via BASS_TRACE=1), run_bass_kernel_spmd imports it. Provide the
    ctypes-based hook so that path works instead of raising."""
    try:
        import antenv.axon_hooks  # noqa: F401

        return
    except ImportError:
        pass
    try:
        from trn_agent_boot.trn_boot import _ntff_profile_via_ctypes

        hook = _ntff_profile_via_ctypes("/opt/axon/libaxon_pjrt.so")
    except Exception:
        hook = None
    mod = types.ModuleType("antenv.axon_hooks")
    mod.get_axon_ntff_profile_hook = lambda: hook
    mod.set_axon_ntff_profile_hook = lambda h: None
    sys.modules["antenv.axon_hooks"] = mod


_install_ntff_hook_shim()


class SlimExitTileContext(tile.TileContext):
    """TileContext whose exit sequence drops the second all-engine barrier.

    The stock epilogue is drain -> barrier -> sem clear -> barrier. The
    first barrier guarantees every engine is idle before the gpsimd range
    sem-clear runs; the trailing barrier only re-synchronizes engines that
    are each about to run off the end of their own queues, so skipping it
    is safe (NRT completion still waits for every queue, and the sem state
    a re-execution needs is restored by the clear).
    """

    def _drain_and_barrier(self, tick_clock, wait_clock):
        drain_inst = self.nc.gpsimd.drain()
        wait_clock.add_sem_waits(
            drain_inst.ins, ScopedClock({None: tick_clock.global_clock})
        )
        assert self.sems is not None
        popped = self.nc._tile_sem_poison_stack.pop()
        assert popped is self._sem_poison
        self.nc.clear_and_free_semaphores(list(self.sems.allocated().values()))

# Problem constants (self-contained; must match the grader's input spec)
B = 16                 # signals in the batch
T = 1048576            # samples per signal
N_FFT = 1024
HOP = 256
N_CORES = 8
SIG_PER_CORE = B // N_CORES   # 2
P = 128                       # SBUF partitions
NBLK = T // HOP               # 4096 hop blocks per signal
CPB = NBLK // P               # 32 output frames per partition
NFRAMES = NBLK                # 4096 output frames per signal

F32 = mybir.dt.float32
AF = mybir.ActivationFunctionType
AX = mybir.AxisListType
ADD = mybir.AluOpType.add
MULT = mybir.AluOpType.mult

# ext01 column base per signal
EXTBASE = [0, 36]

# Per-signal chunk table: name -> (block_offset, n_blocks).
# h0: Pool-squared + Pool-halved (ledge source); h1..h3: ACT-squared +
# Pool-halved; h4, h5: ACT-squared + DVE full reduce; f1 (28,4) per-block
# fused (holds the seam + right-edge sources — it arrives MID-stream so the
# slow partition-shift seam DMAs, ~2.2us for 127 tiny packets, complete
# well before the window adds); f0a/f0b: tiny fused tail chunks.
CHUNKS = {
    "h0": (0, 4),
    "h1": (4, 4),
    "h2": (8, 4),
    "h3": (12, 4),
    "h4": (16, 4),
    "h5": (20, 4),
    "f1": (28, 4),
    "f0a": (24, 2),
    "f0b": (26, 2),
}

# Global arrival order: (sig, chunk). First rides the ACT HWDGE ring (its
# queue opens ~1us before Sync clears its tile-entry drain); the rest ride
# Sync's ring back-to-back.
ARRIVALS = [
    (0, "h1"),   # ACT's first square asap
    (0, "h0"),   # Pool's square + ledge sources
    (0, "h4"),   # DVE's first full reduce
    (1, "h1"),
    (1, "h0"),
    (0, "h2"),
    (1, "h4"),
    (0, "f1"),   # seam sources mid-stream
    (1, "f1"),
    (0, "h5"),
    (1, "h2"),
    (0, "h3"),
    (1, "h5"),
    (1, "h3"),
    (0, "f0a"), (1, "f0a"),
    (0, "f0b"), (1, "f0b"),
]


def build_bass():
    # Bacc (not plain Bass): its compile pipeline splits multi-sem waits
    # into event-semaphore instructions, which this walrus build requires.
    # Bass.__init__ ends with an all-engine barrier whose only job is to
    # order its const-AP memsets against const-AP readers; this kernel
    # reads no const APs (explicit zeros-tile bias), so skip it.
    orig_barrier = bass.Bass.all_engine_barrier
    bass.Bass.all_engine_barrier = lambda self, *, sem_only=False: None
    try:
        nc = bacc.Bacc()
    finally:
        bass.Bass.all_engine_barrier = orig_barrier
    x = nc.dram_tensor("signal", [SIG_PER_CORE, T], F32, kind="ExternalInput")
    y = nc.dram_tensor("out", [SIG_PER_CORE, NFRAMES], F32, kind="ExternalOutput")

    xr = x[:, :].rearrange("b (p f) -> b p f", p=P)   # [2, 128, 8192]
    yr = y[:, :].rearrange("b (p c) -> b p c", p=P)   # [2, 128, 32]

    with SlimExitTileContext(nc) as tc:
        with (
            tc.tile_pool(name="inp", bufs=1) as inp_pool,
            tc.tile_pool(name="sq", bufs=1) as sq_pool,
            tc.tile_pool(name="ext", bufs=1) as ext_pool,
            tc.tile_pool(name="spec", bufs=1) as spec_pool,
            tc.tile_pool(name="small", bufs=1) as small_pool,
        ):
            # --- loads -------------------------------------------------
            tins = {}
            for i, (sig, cn) in enumerate(ARRIVALS):
                b0, nb = CHUNKS[cn]
                t = inp_pool.tile([P, nb * HOP], F32, tag=f"tin_{sig}_{cn}")
                eng = nc.scalar if i == 0 else nc.sync
                eng.dma_start(
                    out=t[:, :], in_=xr[sig, :, b0 * HOP : b0 * HOP + nb * HOP]
                )
                tins[(sig, cn)] = t

            # Zeros tile as the explicit activation bias; dummy Sqrt
            # preloads the ACT table set covering Square+Sqrt up front.
            # Memsets on GpSimd (Memset efficiency 1.0, frees DVE time).
            zb = spec_pool.tile([P, 1], F32, tag="zb")
            nc.gpsimd.memset(zb[:, :], 0.0)
            dummy = spec_pool.tile([1, 1], F32, tag="dummy")
            nc.gpsimd.memset(dummy[0:1, 0:1], 1.0)
            nc.scalar.activation(
                out=dummy[0:1, 0:1], in_=dummy[0:1, 0:1], func=AF.Sqrt,
                bias=zb[0:1, 0:1],
            )

            # Shared block-sum tile for both signals + junk-gap memset
            # (col 35 is read by the windowing as padding; col 71 unused).
            ext = ext_pool.tile([P, 72], F32, tag="ext01")
            nc.gpsimd.memset(ext[:, 35:36], 0.0)

            tsq = {}

            def col(sig, blk):
                return EXTBASE[sig] + 2 + blk

            def act_square(sig, cn):
                b0, nb = CHUNKS[cn]
                t = sq_pool.tile([P, nb * HOP], F32, tag=f"tsq_{sig}_{cn}")
                nc.scalar.activation(
                    out=t[:, :], in_=tins[(sig, cn)][:, :], func=AF.Square,
                    bias=zb[:, 0:1],
                )
                tsq[(sig, cn)] = t

            def pool_square(sig, cn):
                b0, nb = CHUNKS[cn]
                t = sq_pool.tile([P, nb * HOP], F32, tag=f"tsq_{sig}_{cn}")
                tin = tins[(sig, cn)]
                nc.gpsimd.tensor_tensor(
                    out=t[:, :], in0=tin[:, :], in1=tin[:, :], op=MULT
                )
                tsq[(sig, cn)] = t

            def pool_halve(sig, cn):
                b0, nb = CHUNKS[cn]
                hv = sq_pool.tile([P, nb * 128], F32, tag=f"hv_{sig}_{cn}")
                v = tsq[(sig, cn)][:, :].rearrange(
                    "p (g j k) -> p g j k", g=nb, j=2
                )
                nc.gpsimd.tensor_tensor(
                    out=hv[:, :].rearrange("p (g k) -> p g k", g=nb),
                    in0=v[:, :, 0, :], in1=v[:, :, 1, :], op=ADD,
                )
                tsq[(sig, cn, "hv")] = hv

            def dve_halfred(sig, cn):
                b0, nb = CHUNKS[cn]
                nc.vector.tensor_reduce(
                    out=ext[:, col(sig, b0) : col(sig, b0) + nb],
                    in_=tsq[(sig, cn, "hv")][:, :].rearrange(
                        "p (g k) -> p g k", g=nb
                    ),
                    axis=AX.X, op=ADD,
                )

            def dve_fullred(sig, cn):
                b0, nb = CHUNKS[cn]
                nc.vector.tensor_reduce(
                    out=ext[:, col(sig, b0) : col(sig, b0) + nb],
                    in_=tsq[(sig, cn)][:, :].rearrange("p (g k) -> p g k", k=HOP),
                    axis=AX.X, op=ADD,
                )

            def fused_block(sig, cn, k, eng):
                b0, nb = CHUNKS[cn]
                tin = tins[(sig, cn)]
                c = col(sig, b0 + k)
                sl = tin[:, k * HOP : (k + 1) * HOP]
                acc = ext[:, c : c + 1]
                if eng == "act":
                    scr = sq_pool.tile([P, HOP], F32, tag="scr_act", bufs=2)
                    nc.scalar.activation(
                        out=scr[:, :], in_=sl, func=AF.Square,
                        bias=zb[:, 0:1], accum_out=acc,
                    )
                else:
                    scr = sq_pool.tile([P, HOP], F32, tag="scr_dve", bufs=2)
                    nc.vector.scalar_tensor_tensor(
                        out=scr[:, :], in0=sl, scalar=1.0, in1=sl,
                        op0=MULT, op1=MULT, accum_out=acc,
                    )

            SUB = mybir.AluOpType.subtract

            def left_edges(sig):
                # Reflect edges algebraically from block sums already in
                # ext (a 256-sum at an unaligned column costs 3.5x on DVE,
                # so avoid summing entirely):
                #   s_pad[1] = s[0] - x0^2 + x256^2    -> ext col 1
                #   s_pad[0] = s[1] - x256^2 + x512^2  -> ext col 0
                # Tiny aligned strided ops on partition 0 only.
                tin = tins[(sig, "h0")]
                cb = EXTBASE[sig]
                v = tin[0:1, 0:768:256]                # cols {0,256,512}
                s3 = small_pool.tile([P, 3], F32, tag="s3", bufs=2)
                nc.vector.scalar_tensor_tensor(
                    out=s3[0:1, :], in0=v, scalar=1.0, in1=v,
                    op0=MULT, op1=MULT,
                )
                sd = small_pool.tile([P, 2], F32, tag="sd", bufs=2)
                nc.vector.tensor_tensor(
                    out=sd[0:1, :], in0=s3[0:1, 1:3], in1=s3[0:1, 0:2], op=SUB
                )
                nc.vector.tensor_tensor(
                    out=ext[0:1, cb + 1 : cb + 2],
                    in0=ext[0:1, cb + 2 : cb + 3], in1=sd[0:1, 0:1], op=ADD,
                )
                nc.vector.tensor_tensor(
                    out=ext[0:1, cb : cb + 1],
                    in0=ext[0:1, cb + 3 : cb + 4], in1=sd[0:1, 1:2], op=ADD,
                )

            def right_edge_and_seams(sig):
                # s_pad[4098] = s[4095] - x[T-1]^2 + x[T-257]^2 -> ext col
                # 34 (row 127; f1-local cols 767 / 1023 hold those samples).
                # Computed on all partitions (junk rows 0..126 are then
                # overwritten by seam-2 below; row 127 survives).
                tin = tins[(sig, "f1")]
                cb = EXTBASE[sig]
                v = tin[:, 767:1024:256]               # cols {767, 1023}
                s2 = small_pool.tile([P, 2], F32, tag="s2", bufs=2)
                nc.vector.scalar_tensor_tensor(
                    out=s2[:, :], in0=v, scalar=1.0, in1=v,
                    op0=MULT, op1=MULT,
                )
                d2 = small_pool.tile([P, 1], F32, tag="d2", bufs=2)
                nc.vector.tensor_tensor(
                    out=d2[:, :], in0=s2[:, 0:1], in1=s2[:, 1:2], op=SUB
                )
                nc.vector.tensor_tensor(
                    out=ext[:, cb + 34 : cb + 35],
                    in0=ext[:, cb + 33 : cb + 34], in1=d2[:, 0:1], op=ADD,
                )
                # seam1: ext[p, 0:2] = ext[p-1, 32:34]; seam2: ext[p, 34] =
                # ext[p+1, 2] (rows 0..126). On Sync's ring (idle after the
                # bulk triggers).
                nc.sync.dma_start(
                    out=ext[1:128, cb : cb + 2], in_=ext[0:127, cb + 32 : cb + 34]
                )
                nc.sync.dma_start(
                    out=ext[0:127, cb + 34 : cb + 35], in_=ext[1:128, cb + 2 : cb + 3]
                )

            # --- compute, in estimated input-readiness order -----------
            act_square(0, "h1")          # data ~9.3 (ACT ring)
            pool_square(0, "h0")         # data ~10.6
            act_square(0, "h4")          # data ~11.9
            pool_halve(0, "h1")
            dve_halfred(0, "h1")
            act_square(1, "h1")          # data ~13.2
            pool_halve(0, "h0")
            dve_halfred(0, "h0")
            left_edges(0)                # reads ext cols 2/3 (h0 halfred)
            dve_fullred(0, "h4")
            pool_square(1, "h0")         # data ~14.5
            act_square(0, "h2")          # data ~15.8
            pool_halve(1, "h1")
            dve_halfred(1, "h1")
            act_square(1, "h4")          # data ~17.1
            pool_halve(1, "h0")
            dve_halfred(1, "h0")
            left_edges(1)                # reads ext cols 38/39 (h0 halfred)
            # f1 fused (data s0 ~18.4, s1 ~19.7): seam sources (blocks
            # 30/31) first; seams + right edge right after, so the slow
            # partition-shift copies finish mid-stream.
            pool_halve(0, "h2")
            dve_halfred(0, "h2")
            dve_fullred(1, "h4")
            fused_block(0, "f1", 2, "dve")   # block 30
            fused_block(0, "f1", 3, "dve")   # block 31
            fused_block(0, "f1", 0, "act")   # block 28
            fused_block(0, "f1", 1, "dve")   # block 29
            right_edge_and_seams(0)
            fused_block(1, "f1", 2, "dve")
            fused_block(1, "f1", 3, "dve")
            fused_block(1, "f1", 0, "act")
            fused_block(1, "f1", 1, "dve")
            right_edge_and_seams(1)
            act_square(0, "h5")          # data ~21
            dve_fullred(0, "h5")
            act_square(1, "h2")          # data ~22.3
            pool_halve(1, "h2")
            dve_halfred(1, "h2")
            act_square(0, "h3")          # data ~23.6
            pool_halve(0, "h3")
            dve_halfred(0, "h3")
            act_square(1, "h5")          # data ~24.9
            dve_fullred(1, "h5")
            act_square(1, "h3")          # data ~26.2
            pool_halve(1, "h3")
            dve_halfred(1, "h3")
            # tiny fused tails (data ~27.5 .. 29.2), one block per engine
            fused_block(0, "f0a", 0, "act")   # block 24
            fused_block(0, "f0a", 1, "dve")   # block 25
            fused_block(1, "f0a", 0, "act")
            fused_block(1, "f0a", 1, "dve")
            fused_block(0, "f0b", 0, "act")   # block 26
            fused_block(0, "f0b", 1, "dve")   # block 27
            fused_block(1, "f0b", 0, "act")
            fused_block(1, "f0b", 1, "dve")

            # Combined windows on Pool (idle by now): p1/e over the whole
            # 72-wide pair (junk in the 2-col gap, never read downstream),
            # one sqrt on ACT, outputs on Sync.
            p1 = small_pool.tile([P, 70], F32, tag="p1")
            e1 = small_pool.tile([P, 68], F32, tag="e1")
            nc.gpsimd.tensor_tensor(
                out=p1[:, :], in0=ext[:, 0:70], in1=ext[:, 1:71], op=ADD
            )
            nc.gpsimd.tensor_tensor(
                out=e1[:, :], in0=p1[:, 0:68], in1=p1[:, 2:70], op=ADD
            )
            ot = small_pool.tile([P, 68], F32, tag="ot")
            nc.scalar.activation(
                out=ot[:, :], in_=e1[:, :], func=AF.Sqrt, scale=1.0 / N_FFT,
                bias=zb[:, 0:1],
            )
            # Outputs on two different rings so they transfer in parallel.
            nc.sync.dma_start(out=yr[0, :, :], in_=ot[:, 0:32])
            nc.scalar.dma_start(out=yr[1, :, :], in_=ot[:, 36:68])
    nc.finalize()
    return nc


_NC = None


def run(signal: np.ndarray, trace: bool = False):
    global _NC
    sig = np.ascontiguousarray(np.asarray(signal, dtype=np.float32))
    assert sig.shape == (B, T), sig.shape
    if _NC is None:
        _NC = build_bass()
    in_maps = [
        {"signal": np.ascontiguousarray(sig[k * SIG_PER_CORE : (k + 1) * SIG_PER_CORE])}
        for k in range(N_CORES)
    ]
    res = run_bass_kernel_spmd(_NC, in_maps, core_ids=list(range(N_CORES)), trace=trace)
    out = np.concatenate([r["out"] for r in res.results], axis=0)
    return out.reshape(B, NFRAMES, 1).astype(np.float32), res


def kernel(signal: np.ndarray) -> np.ndarray:
    out, _ = run(signal, trace=False)
    return out
